# revision 15
# baseline (speedup 1.0000x reference)
"""FFT depthwise conv == direct 7x7 circular depthwise conv, on 8 TRN2 cores.

out[b,i,j,c] = sum_{u,v} wf[c,u,v] * x[b,(i+u-3)%H,(j+v-3)%W,c],  wf = kernel[:, ::-1, ::-1]

v10: all 49 taps on TensorE via banded-Toeplitz matmuls, CHANNEL-sharded.

Sharding: 24 channels per core, all 8 images. Per channel-pair (2ch x 64
W-window rows on the 128 partitions), the v-convolution is one matmul with
a block-diagonal banded-Toeplitz stationary matrix T[p,m] = wf[c,u,p-m];
the 7 u-taps accumulate in PSUM fp32. Free dim = 2 images x 224 rows
(N=448, PSUM-bank exact), so only 1344 matmuls/core at ~190ns back-to-back
gap with LDWEIGHTS fully hidden. Input host-side transposed per channel
(partitions = padded W, free = padded H): tap u is a free-dim offset.

Channel-sharding inverts the startup supply/burn ratio of the batch-
sharded variant: one pair costs ~21us of PE time but only ~0.7MB of
input+weights, so after a single fine-grained pair-0 load the DMA
pipeline can never starve the PE again. DVE does all psum->bf16 copies;
host does every layout (un)shuffle.

DMA granularity: descriptor batches sized so the 16-engine pool fans out
(~64 descriptors/engine round-robin): pair-0 x lands as 4 per-window
1024x460B batches; steady-state x moves as 2-pair 1024x3680B batches,
out as per-pair 512x3584B batches, w as small per-pair batches.
"""

import os
import sys

for _p in ("/opt/trn_rl_repo", "/root/.axon_site/_ro/trn_rl_repo"):
    if os.path.isdir(_p) and _p not in sys.path:
        sys.path.insert(0, _p)

import numpy as np

import concourse.bacc as bacc
import concourse.bass as bass
import concourse.mybir as mybir
from concourse.bass_utils import run_bass_kernel_spmd
from concourse.tile import TileContext

F32 = mybir.dt.float32
BF16 = mybir.dt.bfloat16

B, H, W, C, K = 8, 224, 224, 192, 7
NCORES = 8
PAD = K // 2                  # 3
HP = H + 2 * PAD              # 230 padded rows (free dim)
WP = W + 2 * PAD              # 230 padded cols (partition windows)
NPAIR_G = C // 2              # 96 global channel pairs
NPAIR = NPAIR_G // NCORES     # 12 pairs per core
W0S = [0, 58, 116, 166]       # window starts (input w-padded coords)
M0S = [0, 0, 0, 8]            # first valid m per window
NJS = [58, 58, 58, 50]        # valid rows per window
NT = len(W0S)                 # 4 windows
NIB = B // 2                  # 4 image-pairs per matmul group (N = 2*224)

# DVE tap-offload: for chosen (t, ib) groups, DVE computes the even-u taps
# (free-dim offset stays 4B-aligned -> 4x/2x perf modes) as bf16 MAC chains
# while PE only accumulates the odd-u taps in PSUM; an stt merges the two.
DVE_TAPS = [(u, v) for u in (0, 2, 4, 6) for v in range(K)]
PE_U_OFF = [1, 3, 5]
OFFLOAD = {q: {(3, 3)} for q in range(11)}
for _q in (3, 5, 7):
    OFFLOAD[_q].add((2, 3))
OFFLOAD[11] = set()

# strides (elements) of the DRAM tensors
XS_P, XS_T, XS_PT = 8 * HP, 128 * 8 * HP, NT * 128 * 8 * HP
WS_P, WS_PR = K * 128, 128 * K * 128
OS_P, OS_T, OS_PR = NT * B * H, B * H, 128 * NT * B * H


def build_nc():
    nc = bacc.Bacc()
    # x: [pair, t, p, img, hp]   w: [pair, p, u, m]   out: [pair, p, t, img, i]
    x_d = nc.declare_dram_parameter("x", [NPAIR, NT, 128, B, HP], BF16, isOutput=False)
    wv_d = nc.declare_dram_parameter("wv", [128, NPAIR * K * K], F32, isOutput=False)
    w_d = nc.declare_dram_parameter("w", [NPAIR, 128, K, 128], BF16, isOutput=False)
    o_d = nc.declare_dram_parameter("out", [NPAIR, 128, NT, B, H], BF16, isOutput=True)
    xh = x_d.tensor if hasattr(x_d, "tensor") else x_d
    wh = w_d.tensor if hasattr(w_d, "tensor") else w_d
    oh = o_d.tensor if hasattr(o_d, "tensor") else o_d
    wvh = wv_d.tensor if hasattr(wv_d, "tensor") else wv_d
    mult = mybir.AluOpType.mult
    add = mybir.AluOpType.add

    with TileContext(nc) as tc:
        with (
            tc.tile_pool(name="ramp", bufs=1) as rpool,
            tc.tile_pool(name="xin", bufs=3) as xpool,
            tc.tile_pool(name="wts", bufs=3) as wpool,
            tc.tile_pool(name="outp", bufs=3) as opool,
            tc.tile_pool(name="dvp", bufs=2) as dpool,
            tc.tile_pool(name="psum", bufs=8, space="PSUM") as ppool,
        ):
            # per-partition tap weights for the DVE-offloaded groups
            wvec = rpool.tile([128, NPAIR * K * K], F32, name="wvec", tag="wvec")
            nc.scalar.dma_start(
                out=wvec[:, :],
                in_=bass.AP(wvh, 0, [[NPAIR * K * K, 128], [1, NPAIR * K * K]]),
            )

            # pre-warm the PE clock gate (HAM) with throwaway matmuls so
            # the real stream starts at 2.4 GHz instead of 1.2 GHz
            warm = rpool.tile([128, 128], BF16, name="warm", tag="warm")
            nc.vector.memset(warm[:], 1.0)
            wps = ppool.tile([128, 512], F32, name="warmps", tag="ps")
            NWARM = 62
            for i in range(NWARM):
                nc.tensor.matmul(
                    wps[0:128, 0:128],
                    warm[:, 0:128],
                    warm[:, 0:128],
                    start=(i == 0),
                    stop=(i == NWARM - 1),
                )

            def compute_pair(q, rhs_fn, w_fn, ot, fine_out):
                """All 16 psum groups of pair q + copies + out DMA(s)."""
                for t in range(NT):
                    for ib in range(NIB):
                        offl = (t, ib) in OFFLOAD.get(q, ())
                        pe_us = PE_U_OFF if offl else range(K)
                        ps = ppool.tile(
                            [128, 512], F32, name=f"ps{q}_{t}_{ib}", tag="ps"
                        )
                        for ui, u in enumerate(pe_us):
                            nc.tensor.matmul(
                                ps[:, 0:2 * H],
                                w_fn(u),
                                rhs_fn(t, ib, u),
                                start=(ui == 0),
                                stop=(ui == len(pe_us) - 1),
                            )
                        ps3 = ps[:, 0:2 * H].rearrange("p (b i) -> p b i", b=2)
                        if offl:
                            # DVE path needs partition-base-0 reads (bases
                            # must be 32-aligned), so DMA a tile holding all
                            # 7 v-shifted copies of this group's window rows
                            # straight from DRAM -- the v-shift becomes a
                            # free-dim offset. Rows >=122 read junk; unused.
                            xv = dpool.tile(
                                [128, K, 2, HP], BF16,
                                name=f"xv{q}_{t}_{ib}", tag="xv",
                            )
                            nc.sync.dma_start(
                                out=xv[:, :, :, :],
                                in_=bass.AP(
                                    xh,
                                    q * XS_PT + t * XS_T + 2 * ib * HP,
                                    [[XS_P, 128], [XS_P, K], [HP, 2], [1, HP]],
                                ),
                            )
                            acc = dpool.tile(
                                [128, 2, H], BF16, name=f"ac{q}_{t}_{ib}", tag="acc"
                            )
                            tmps = [
                                dpool.tile(
                                    [128, 2, H], BF16,
                                    name=f"tm{q}_{t}_{ib}_{j}", tag=f"tmp{j}",
                                )
                                for j in range(2)
                            ]
                            u0, v0 = DVE_TAPS[0]
                            c0 = (q * K + u0) * K + v0
                            nc.vector.tensor_scalar(
                                acc[:, :, :],
                                xv[:, v0, :, u0:u0 + H],
                                wvec[:, c0:c0 + 1],
                                None,
                                mult,
                            )
                            for j, (u, v) in enumerate(DVE_TAPS[1:]):
                                cc = (q * K + u) * K + v
                                tmp = tmps[j % 2]
                                nc.vector.tensor_scalar(
                                    tmp[:, :, :],
                                    xv[:, v, :, u:u + H],
                                    wvec[:, cc:cc + 1],
                                    None,
                                    mult,
                                )
                                nc.vector.tensor_tensor(
                                    acc[:, :, :], acc[:, :, :], tmp[:, :, :], add
                                )
                            nc.vector.scalar_tensor_tensor(
                                ot[:, t, 2 * ib:2 * ib + 2, :],
                                ps3,
                                1.0,
                                acc[:, :, :],
                                mult,
                                add,
                            )
                        else:
                            nc.scalar.copy(
                                out=ot[:, t, 2 * ib:2 * ib + 2, :], in_=ps3
                            )
                        if fine_out:
                            # drain per group: the last DMA after the last
                            # copy is only ~115KB (256 x 448B descriptors)
                            nc.scalar.dma_start(
                                out=bass.AP(
                                    oh,
                                    q * OS_PR + t * OS_T + 2 * ib * H,
                                    [[OS_P, 128], [H, 2], [1, H]],
                                ),
                                in_=ot[:, t, 2 * ib:2 * ib + 2, :],
                            )
                if not fine_out:
                    # whole pair: 512 x 3584B descriptors
                    nc.scalar.dma_start(
                        out=bass.AP(
                            oh, q * OS_PR, [[OS_P, 128], [OS_T, NT], [1, B * H]]
                        ),
                        in_=ot[:, :, :, :],
                    )

            # ---- pair 0: fine-grained ramp (one tile per window) ----
            rxt = []
            for t in range(NT):
                rx = rpool.tile([128, B, HP], BF16, name=f"rx{t}", tag=f"rx{t}")
                nc.sync.dma_start(
                    out=rx[:, :, :],
                    in_=bass.AP(xh, t * XS_T, [[XS_P, 128], [HP, B], [1, HP]]),
                )
                rxt.append(rx)
            rw = rpool.tile([128, K, 128], BF16, name="rw", tag="rw")
            nc.gpsimd.dma_start(
                out=rw[:, :, :],
                in_=bass.AP(wh, 0, [[WS_P, 128], [128, K], [1, 128]]),
            )
            rot = rpool.tile([128, NT, B, H], BF16, name="ro", tag="ro")
            compute_pair(
                0,
                lambda t, ib, u: rxt[t][:, 2 * ib:2 * ib + 2, u:u + H],
                lambda u: rw[:, u, :],
                rot,
                fine_out=True,
            )

            # ---- pairs 1..11: 2-pair coarse blocks (last pair solo) ----
            blocks = [(1, 2), (3, 2), (5, 2), (7, 2), (9, 2), (11, 1)]
            for q0, n in blocks:
                xt = xpool.tile([128, 2, NT, B, HP], BF16, name=f"x{q0}", tag="x")
                nc.sync.dma_start(
                    out=xt[:, 0:n, :, :, :],
                    in_=bass.AP(
                        xh,
                        q0 * XS_PT,
                        [[XS_P, 128], [XS_PT, n], [XS_T, NT], [1, B * HP]],
                    ),
                )
                wt = wpool.tile([128, 2, K, 128], BF16, name=f"w{q0}", tag="w")
                nc.gpsimd.dma_start(
                    out=wt[:, 0:n, :, :],
                    in_=bass.AP(
                        wh, q0 * WS_PR, [[WS_P, 128], [WS_PR, n], [1, K * 128]]
                    ),
                )
                for pl in range(n):
                    q = q0 + pl
                    ot = opool.tile([128, NT, B, H], BF16, name=f"o{q}", tag="o")
                    compute_pair(
                        q,
                        lambda t, ib, u, pl=pl: xt[
                            :, pl, t, 2 * ib:2 * ib + 2, u:u + H
                        ],
                        lambda u, pl=pl: wt[:, pl, u, :],
                        ot,
                        fine_out=(q == NPAIR - 1),
                    )
    return nc


def _host_x(x):
    """x: (B, H, W, C) f32 -> (NCORES, NPAIR, NT, 128, B, HP) bf16."""
    import ml_dtypes

    # xT[b, c, wp, hp] = x[b, (hp-3)%H, (wp-3)%W, c]
    xT = np.ascontiguousarray(x.transpose(0, 3, 2, 1))          # (B, C, W, H)
    xT = np.pad(xT, ((0, 0), (0, 0), (PAD, PAD), (PAD, PAD)), mode="wrap")
    xT = xT.astype(ml_dtypes.bfloat16)                          # (B, C, WP, HP)
    gx = np.empty((NPAIR_G, NT, 128, B, HP), dtype=ml_dtypes.bfloat16)
    for t, w0 in enumerate(W0S):
        win = xT[:, :, w0:w0 + 64, :].transpose(1, 2, 0, 3)     # (C, 64, B, HP)
        gx[:, t, 0:64] = win[0::2]
        gx[:, t, 64:128] = win[1::2]
    return gx.reshape(NCORES, NPAIR, NT, 128, B, HP)


def _host_w(kernel):
    """kernel: (C, K, K) -> (NCORES, NPAIR, 128, K, 128) bf16 block-diag."""
    import ml_dtypes

    wf = kernel[:, ::-1, ::-1].astype(np.float32)               # flipped taps
    wdev = np.zeros((NPAIR_G, 128, K, 128), dtype=np.float32)
    m = np.arange(64)
    for v in range(K):
        pm = m + v                                              # p = m + v
        ok = pm < 64
        # advanced indices (pm, m) land in front: result (ndiag, NPAIR_G, K)
        wdev[:, pm[ok], :, m[ok]] = wf[0::2, :, v][None, :, :]
        wdev[:, 64 + pm[ok], :, 64 + m[ok]] = wf[1::2, :, v][None, :, :]
    return wdev.reshape(NCORES, NPAIR, 128, K, 128).astype(ml_dtypes.bfloat16)


def _host_wvec(kernel):
    """kernel: (C, K, K) -> (NCORES, 128, NPAIR*K*K) f32 per-partition taps."""
    wf = kernel[:, ::-1, ::-1].astype(np.float32).reshape(NCORES, NPAIR, 2, K * K)
    wv = np.empty((NCORES, 128, NPAIR, K * K), dtype=np.float32)
    wv[:, 0:64] = wf[:, None, :, 0, :]
    wv[:, 64:128] = wf[:, None, :, 1, :]
    return np.ascontiguousarray(wv.reshape(NCORES, 128, NPAIR * K * K))


def _host_unshuffle(odev):
    """odev: (NCORES, NPAIR, 128, NT, B, H) -> (B, H, W, C) f32."""
    o = np.asarray(odev, dtype=np.float32).reshape(NPAIR_G, 128, NT, B, H)
    out = np.empty((B, H, W, C), dtype=np.float32)
    for t, w0 in enumerate(W0S):
        m0, nj = M0S[t], NJS[t]
        j0 = w0 + m0
        # o[g, m, t, img, i] -> out[img, i, j0+mm, 2g (+1 for p>=64)]
        out[:, :, j0:j0 + nj, 0::2] = o[:, m0:m0 + nj, t].transpose(2, 3, 1, 0)
        out[:, :, j0:j0 + nj, 1::2] = o[:, 64 + m0:64 + m0 + nj, t].transpose(
            2, 3, 1, 0
        )
    return out


_NC_CACHE = {}


def _get_nc():
    if "nc" not in _NC_CACHE:
        nc = build_nc()
        nc.finalize()
        _NC_CACHE["nc"] = nc
    return _NC_CACHE["nc"]


def run(x, kernel, trace=False, **kw):
    assert x.shape == (B, H, W, C) and kernel.shape == (C, K, K)
    nc = _get_nc()
    xdev = _host_x(np.asarray(x, dtype=np.float32))
    wdev = _host_w(np.asarray(kernel))
    wvec = _host_wvec(np.asarray(kernel))
    in_maps = [
        {"x": xdev[b], "w": wdev[b], "wv": wvec[b]} for b in range(NCORES)
    ]
    res = run_bass_kernel_spmd(nc, in_maps, list(range(NCORES)), trace=trace, **kw)
    odev = np.stack([np.asarray(res.results[b]["out"]) for b in range(NCORES)])
    return _host_unshuffle(odev), res


def kernel(x, kernel):
    out, _ = run(np.asarray(x), np.asarray(kernel))
    return out
